# revision 12
# baseline (speedup 1.0000x reference)
"""Trainium2 Bass kernel for nn_ConditionedDense (hypernetwork-conditioned dense).

Reference computation:
    A = einsum('bnp,pq->bnq', P, Wk)         # hypernetwork: per-position weights
    W = relu(A).reshape(B, N, c_in, c_out)
    out = einsum('bni,bnio->bno', X, W)

Strategy: pure data parallel over 8 NeuronCores (shard batch dim). Per core
16384 positions, tiled 128 positions/tile, 4 tiles per DMA chunk:
  - PE matmul computes A-tile [128 pos, 1024] in PSUM (lhsT = P^T tile,
    rhs = Wk, both bf16; Wk host-permuted to q = o*32+i layout)
  - ACT applies relu (PSUM -> SBUF, bf16 out)
  - DVE (and GPSIMD for a fraction of tiles) multiplies by X broadcast
    over o; DVE grouped-reduces over i (innermost) and upcasts to fp32
Host side (free): P transposed per shard, Wk column-permuted, X/P/Wk cast
to bf16.
"""

import os
from contextlib import ExitStack

import numpy as np
import ml_dtypes

import concourse.bass as bass
import concourse.tile as tile
from concourse import bacc, mybir
from concourse.bass_utils import run_bass_kernel_spmd

C_IN = 32
C_OUT = 32
P_DIM = 64
Q = C_IN * C_OUT  # 1024
B, N = 32, 4096
N_CORES = 8
B_SH = B // N_CORES          # 4 batches per core
NPOS = B_SH * N              # 16384 positions per core
TILE_P = 128                 # positions per tile
N_TILES = NPOS // TILE_P     # 128
CHUNK = 8                    # tiles per DMA chunk
N_CHUNKS = N_TILES // CHUNK  # 16

F32 = mybir.dt.float32
BF16 = mybir.dt.bfloat16

_BUILD_CACHE = {}
LAST_RESULTS = None  # BassKernelResults of the most recent run (for profiling)


def _build_nc():
    nc = bacc.Bacc(
        "TRN2", target_bir_lowering=False, debug=False, num_devices=N_CORES
    )
    X_d = nc.declare_dram_parameter("X", [NPOS, C_IN], BF16, isOutput=False)
    PT_d = nc.declare_dram_parameter("PT", [P_DIM, NPOS], BF16, isOutput=False)
    Wk_d = nc.declare_dram_parameter("Wk", [P_DIM, Q], BF16, isOutput=False)
    out_d = nc.declare_dram_parameter("out", [NPOS, C_OUT], BF16, isOutput=True)

    relu = mybir.ActivationFunctionType.Relu
    mult = mybir.AluOpType.mult
    add = mybir.AluOpType.add

    with ExitStack() as ctx:
        tc = ctx.enter_context(tile.TileContext(nc))
        wkp = ctx.enter_context(tc.tile_pool(name="wk", bufs=1))
        xp = ctx.enter_context(tc.tile_pool(name="x", bufs=3))
        pp = ctx.enter_context(tc.tile_pool(name="pT", bufs=3))
        apool = ctx.enter_context(tc.tile_pool(name="apsum", bufs=4, space="PSUM"))
        wp = ctx.enter_context(tc.tile_pool(name="w", bufs=4))
        mp = ctx.enter_context(tc.tile_pool(name="m", bufs=4))
        t1p = ctx.enter_context(tc.tile_pool(name="t1", bufs=4))
        t2p = ctx.enter_context(tc.tile_pool(name="t2", bufs=4))
        op = ctx.enter_context(tc.tile_pool(name="o", bufs=3))

        wk_t = wkp.tile([P_DIM, Q], BF16)
        nc.sync.dma_start(out=wk_t[:], in_=Wk_d[:])

        for ch in range(N_CHUNKS):
            # chunk loads: CHUNK * 128 positions per DMA
            x_c = xp.tile([TILE_P, CHUNK, C_IN], BF16)
            nc.sync.dma_start(
                out=x_c[:],
                in_=X_d[bass.ts(ch, TILE_P * CHUNK), :].rearrange(
                    "(a p) i -> p a i", p=TILE_P
                ),
            )
            pT_c = pp.tile([P_DIM, CHUNK * TILE_P], BF16)
            nc.sync.dma_start(
                out=pT_c[:], in_=PT_d[:, bass.ts(ch, TILE_P * CHUNK)]
            )
            o_c = op.tile([TILE_P, CHUNK, C_OUT], BF16)

            for j in range(CHUNK):
                a_t = apool.tile([TILE_P, Q], F32)
                lhsT = pT_c[:, bass.ts(j, TILE_P)]
                nc.tensor.matmul(
                    a_t[:, 0:512], lhsT=lhsT, rhs=wk_t[:, 0:512],
                    start=True, stop=True,
                )
                nc.tensor.matmul(
                    a_t[:, 512:1024], lhsT=lhsT, rhs=wk_t[:, 512:1024],
                    start=True, stop=True,
                )

                # relu: PSUM -> SBUF, cast to bf16 (ACT engine)
                w_t = wp.tile([TILE_P, Q], BF16)
                nc.scalar.activation(w_t[:], a_t[:], relu)

                # m[p, o, i] = w[p, o, i] * x[p, i]   (DVE, 2x bf16)
                m_t = mp.tile([TILE_P, Q], BF16)
                w3 = w_t[:].rearrange("p (o i) -> p o i", o=C_OUT)
                m3 = m_t[:].rearrange("p (o i) -> p o i", o=C_OUT)
                x3 = x_c[:, j, :].unsqueeze(1).broadcast_to(
                    [TILE_P, C_OUT, C_IN]
                )
                nc.vector.tensor_tensor(out=m3, in0=w3, in1=x3, op=mult)

                # Reduce over i (innermost, 32 wide). TensorReduce has no
                # 2x DVE mode, so do the first halving as 2x TENSOR_TENSOR
                # adds (L1 on GPSIMD, L2 on DVE), then one short reduce.
                t1 = t1p.tile([TILE_P, C_OUT, 16], BF16)
                nc.gpsimd.tensor_tensor(
                    out=t1[:], in0=m3[:, :, 0:16], in1=m3[:, :, 16:32], op=add
                )
                t2 = t2p.tile([TILE_P, C_OUT, 8], BF16)
                nc.vector.tensor_tensor(
                    out=t2[:], in0=t1[:, :, 0:8], in1=t1[:, :, 8:16], op=add
                )
                with nc.allow_low_precision("bf16 reduce, fp32 internal accum"):
                    nc.vector.tensor_reduce(
                        out=o_c[:, j, :], in_=t2[:], axis=mybir.AxisListType.X,
                        op=add,
                    )

            nc.sync.dma_start(
                out=out_d[bass.ts(ch, TILE_P * CHUNK), :].rearrange(
                    "(a p) i -> p a i", p=TILE_P
                ),
                in_=o_c[:],
            )

    nc.finalize()
    return nc


def _get_nc():
    key = "v2"
    if key not in _BUILD_CACHE:
        _BUILD_CACHE[key] = _build_nc()
    return _BUILD_CACHE[key]


def kernel(X, P, Wk):
    global LAST_RESULTS
    X = np.asarray(X, dtype=np.float32)
    P = np.asarray(P, dtype=np.float32)
    Wk = np.asarray(Wk, dtype=np.float32)
    bf16 = ml_dtypes.bfloat16

    # Host-side prep (free): shard, transpose P, permute Wk columns so the
    # device-side layout is q = o*32 + i; cast matmul operands to bf16.
    WkP = np.ascontiguousarray(
        Wk.reshape(P_DIM, C_IN, C_OUT).transpose(0, 2, 1).reshape(P_DIM, Q)
    ).astype(bf16)
    in_maps = []
    for c in range(N_CORES):
        Xc = np.ascontiguousarray(
            X[c * B_SH:(c + 1) * B_SH].reshape(NPOS, C_IN)
        ).astype(bf16)
        PTc = np.ascontiguousarray(
            P[c * B_SH:(c + 1) * B_SH].reshape(NPOS, P_DIM).T
        ).astype(bf16)
        in_maps.append({"X": Xc, "PT": PTc, "Wk": WkP})

    nc = _get_nc()
    trace = os.environ.get("BASS_PROFILE", "0") == "1"
    kw = {}
    if os.environ.get("BASS_TMPDIR"):
        kw["tmpdir"] = os.environ["BASS_TMPDIR"]
    res = run_bass_kernel_spmd(
        nc, in_maps, list(range(N_CORES)), trace=trace, **kw
    )
    LAST_RESULTS = res

    out = np.empty((B, N, C_OUT), dtype=np.float32)
    for c in range(N_CORES):
        out[c * B_SH:(c + 1) * B_SH] = (
            np.asarray(res.results[c]["out"])
            .astype(np.float32)
            .reshape(B_SH, N, C_OUT)
        )
    return out


# revision 13
# speedup vs baseline: 1.3733x; 1.3733x over previous
"""Trainium2 Bass kernel for nn_ConditionedDense (hypernetwork-conditioned dense).

Reference computation:
    A = einsum('bnp,pq->bnq', P, Wk)         # hypernetwork: per-position weights
    W = relu(A).reshape(B, N, c_in, c_out)
    out = einsum('bni,bnio->bno', X, W)

Strategy: pure data parallel over 8 NeuronCores (shard batch dim). Per core
16384 positions, tiled 128 positions/tile, 4 tiles per DMA chunk:
  - PE matmul computes A-tile [128 pos, 1024] in PSUM (lhsT = P^T tile,
    rhs = Wk, both bf16; Wk host-permuted to q = o*32+i layout)
  - ACT applies relu (PSUM -> SBUF, bf16 out)
  - DVE (and GPSIMD for a fraction of tiles) multiplies by X broadcast
    over o; DVE grouped-reduces over i (innermost) and upcasts to fp32
Host side (free): P transposed per shard, Wk column-permuted, X/P/Wk cast
to bf16.
"""

import os
from contextlib import ExitStack

import numpy as np
import ml_dtypes

import concourse.bass as bass
import concourse.tile as tile
from concourse import bacc, mybir
from concourse.bass_utils import run_bass_kernel_spmd

C_IN = 32
C_OUT = 32
P_DIM = 64
Q = C_IN * C_OUT  # 1024
B, N = 32, 4096
N_CORES = 8
B_SH = B // N_CORES          # 4 batches per core
NPOS = B_SH * N              # 16384 positions per core
TILE_P = 128                 # positions per tile
N_TILES = NPOS // TILE_P     # 128
CHUNK = 8                    # tiles per DMA chunk
N_CHUNKS = N_TILES // CHUNK  # 16

F32 = mybir.dt.float32
BF16 = mybir.dt.bfloat16

_BUILD_CACHE = {}
LAST_RESULTS = None  # BassKernelResults of the most recent run (for profiling)


def _build_nc():
    nc = bacc.Bacc(
        "TRN2", target_bir_lowering=False, debug=False, num_devices=N_CORES
    )
    X_d = nc.declare_dram_parameter("X", [NPOS, C_IN], BF16, isOutput=False)
    PT_d = nc.declare_dram_parameter("PT", [P_DIM, NPOS], BF16, isOutput=False)
    Wk_d = nc.declare_dram_parameter("Wk", [P_DIM, Q], BF16, isOutput=False)
    out_d = nc.declare_dram_parameter("out", [NPOS, C_OUT], BF16, isOutput=True)

    relu = mybir.ActivationFunctionType.Relu
    mult = mybir.AluOpType.mult
    add = mybir.AluOpType.add

    with ExitStack() as ctx:
        tc = ctx.enter_context(tile.TileContext(nc))
        wkp = ctx.enter_context(tc.tile_pool(name="wk", bufs=1))
        xp = ctx.enter_context(tc.tile_pool(name="x", bufs=3))
        pp = ctx.enter_context(tc.tile_pool(name="pT", bufs=3))
        apool = ctx.enter_context(tc.tile_pool(name="apsum", bufs=4, space="PSUM"))
        wp = ctx.enter_context(tc.tile_pool(name="w", bufs=4))
        mp = ctx.enter_context(tc.tile_pool(name="m", bufs=4))
        t1p = ctx.enter_context(tc.tile_pool(name="t1", bufs=4))
        t2p = ctx.enter_context(tc.tile_pool(name="t2", bufs=4))
        op = ctx.enter_context(tc.tile_pool(name="o", bufs=3))

        wk_t = wkp.tile([P_DIM, Q], BF16)
        nc.sync.dma_start(out=wk_t[:], in_=Wk_d[:])

        for ch in range(N_CHUNKS):
            # chunk loads: CHUNK * 128 positions per DMA
            x_c = xp.tile([TILE_P, CHUNK, C_IN], BF16)
            nc.sync.dma_start(
                out=x_c[:],
                in_=X_d[bass.ts(ch, TILE_P * CHUNK), :].rearrange(
                    "(a p) i -> p a i", p=TILE_P
                ),
            )
            pT_c = pp.tile([P_DIM, CHUNK * TILE_P], BF16)
            nc.sync.dma_start(
                out=pT_c[:], in_=PT_d[:, bass.ts(ch, TILE_P * CHUNK)]
            )
            o_c = op.tile([TILE_P, CHUNK, C_OUT], BF16)

            for j in range(CHUNK):
                a_t = apool.tile([TILE_P, Q], F32)
                lhsT = pT_c[:, bass.ts(j, TILE_P)]
                nc.tensor.matmul(
                    a_t[:, 0:512], lhsT=lhsT, rhs=wk_t[:, 0:512],
                    start=True, stop=True,
                )
                nc.tensor.matmul(
                    a_t[:, 512:1024], lhsT=lhsT, rhs=wk_t[:, 512:1024],
                    start=True, stop=True,
                )

                # relu: PSUM -> SBUF, cast to bf16 (ACT engine)
                w_t = wp.tile([TILE_P, Q], BF16)
                nc.scalar.activation(w_t[:], a_t[:], relu)

                # m[p, o, i] = w[p, o, i] * x[p, i]   (DVE, 2x bf16)
                m_t = mp.tile([TILE_P, Q], BF16)
                w3 = w_t[:].rearrange("p (o i) -> p o i", o=C_OUT)
                m3 = m_t[:].rearrange("p (o i) -> p o i", o=C_OUT)
                x3 = x_c[:, j, :].unsqueeze(1).broadcast_to(
                    [TILE_P, C_OUT, C_IN]
                )
                nc.vector.tensor_tensor(out=m3, in0=w3, in1=x3, op=mult)

                # Reduce over i (innermost, 32 wide). TensorReduce has no
                # 2x DVE mode, so do the first halving as 2x TENSOR_TENSOR
                # adds (L1 on GPSIMD, L2 on DVE), then one short reduce.
                t1 = t1p.tile([TILE_P, C_OUT, 16], BF16)
                nc.vector.tensor_tensor(
                    out=t1[:], in0=m3[:, :, 0:16], in1=m3[:, :, 16:32], op=add
                )
                t2 = t2p.tile([TILE_P, C_OUT, 8], BF16)
                nc.vector.tensor_tensor(
                    out=t2[:], in0=t1[:, :, 0:8], in1=t1[:, :, 8:16], op=add
                )
                with nc.allow_low_precision("bf16 reduce, fp32 internal accum"):
                    nc.vector.tensor_reduce(
                        out=o_c[:, j, :], in_=t2[:], axis=mybir.AxisListType.X,
                        op=add,
                    )

            nc.sync.dma_start(
                out=out_d[bass.ts(ch, TILE_P * CHUNK), :].rearrange(
                    "(a p) i -> p a i", p=TILE_P
                ),
                in_=o_c[:],
            )

    nc.finalize()
    return nc


def _get_nc():
    key = "v2"
    if key not in _BUILD_CACHE:
        _BUILD_CACHE[key] = _build_nc()
    return _BUILD_CACHE[key]


def kernel(X, P, Wk):
    global LAST_RESULTS
    X = np.asarray(X, dtype=np.float32)
    P = np.asarray(P, dtype=np.float32)
    Wk = np.asarray(Wk, dtype=np.float32)
    bf16 = ml_dtypes.bfloat16

    # Host-side prep (free): shard, transpose P, permute Wk columns so the
    # device-side layout is q = o*32 + i; cast matmul operands to bf16.
    WkP = np.ascontiguousarray(
        Wk.reshape(P_DIM, C_IN, C_OUT).transpose(0, 2, 1).reshape(P_DIM, Q)
    ).astype(bf16)
    in_maps = []
    for c in range(N_CORES):
        Xc = np.ascontiguousarray(
            X[c * B_SH:(c + 1) * B_SH].reshape(NPOS, C_IN)
        ).astype(bf16)
        PTc = np.ascontiguousarray(
            P[c * B_SH:(c + 1) * B_SH].reshape(NPOS, P_DIM).T
        ).astype(bf16)
        in_maps.append({"X": Xc, "PT": PTc, "Wk": WkP})

    nc = _get_nc()
    trace = os.environ.get("BASS_PROFILE", "0") == "1"
    kw = {}
    if os.environ.get("BASS_TMPDIR"):
        kw["tmpdir"] = os.environ["BASS_TMPDIR"]
    res = run_bass_kernel_spmd(
        nc, in_maps, list(range(N_CORES)), trace=trace, **kw
    )
    LAST_RESULTS = res

    out = np.empty((B, N, C_OUT), dtype=np.float32)
    for c in range(N_CORES):
        out[c * B_SH:(c + 1) * B_SH] = (
            np.asarray(res.results[c]["out"])
            .astype(np.float32)
            .reshape(B_SH, N, C_OUT)
        )
    return out


# revision 15
# speedup vs baseline: 1.4844x; 1.0809x over previous
"""Trainium2 Bass kernel for nn_ConditionedDense (hypernetwork-conditioned dense).

Reference computation:
    A = einsum('bnp,pq->bnq', P, Wk)         # hypernetwork: per-position weights
    W = relu(A).reshape(B, N, c_in, c_out)
    out = einsum('bni,bnio->bno', X, W)

Strategy: pure data parallel over 8 NeuronCores (shard batch dim). Per core
16384 positions, tiled 128 positions/tile, 4 tiles per DMA chunk:
  - PE matmul computes A-tile [128 pos, 1024] in PSUM (lhsT = P^T tile,
    rhs = Wk, both bf16; Wk host-permuted to q = o*32+i layout)
  - ACT applies relu (PSUM -> SBUF, bf16 out)
  - DVE (and GPSIMD for a fraction of tiles) multiplies by X broadcast
    over o; DVE grouped-reduces over i (innermost) and upcasts to fp32
Host side (free): P transposed per shard, Wk column-permuted, X/P/Wk cast
to bf16.
"""

import os
from contextlib import ExitStack

import numpy as np
import ml_dtypes

import concourse.bass as bass
import concourse.tile as tile
from concourse import bacc, mybir
from concourse.bass_utils import run_bass_kernel_spmd

C_IN = 32
C_OUT = 32
P_DIM = 64
Q = C_IN * C_OUT  # 1024
B, N = 32, 4096
N_CORES = 8
B_SH = B // N_CORES          # 4 batches per core
NPOS = B_SH * N              # 16384 positions per core
TILE_P = 128                 # positions per tile
N_TILES = NPOS // TILE_P     # 128
CHUNK = 8                    # tiles per DMA chunk
N_CHUNKS = N_TILES // CHUNK  # 16

F32 = mybir.dt.float32
BF16 = mybir.dt.bfloat16

_BUILD_CACHE = {}
LAST_RESULTS = None  # BassKernelResults of the most recent run (for profiling)


def _build_nc():
    nc = bacc.Bacc(
        "TRN2", target_bir_lowering=False, debug=False, num_devices=N_CORES
    )
    X_d = nc.declare_dram_parameter("X", [NPOS, C_IN], BF16, isOutput=False)
    PT_d = nc.declare_dram_parameter("PT", [P_DIM, NPOS], BF16, isOutput=False)
    Wk_d = nc.declare_dram_parameter("Wk", [P_DIM, Q], BF16, isOutput=False)
    out_d = nc.declare_dram_parameter("out", [NPOS, C_OUT], BF16, isOutput=True)

    relu = mybir.ActivationFunctionType.Relu
    mult = mybir.AluOpType.mult
    add = mybir.AluOpType.add

    with ExitStack() as ctx:
        tc = ctx.enter_context(tile.TileContext(nc))
        wkp = ctx.enter_context(tc.tile_pool(name="wk", bufs=1))
        xp = ctx.enter_context(tc.tile_pool(name="x", bufs=3))
        pp = ctx.enter_context(tc.tile_pool(name="pT", bufs=3))
        apool = ctx.enter_context(tc.tile_pool(name="apsum", bufs=2, space="PSUM"))
        wp = ctx.enter_context(tc.tile_pool(name="w", bufs=4))
        mp = ctx.enter_context(tc.tile_pool(name="m", bufs=4))
        t1p = ctx.enter_context(tc.tile_pool(name="t1", bufs=4))
        t2p = ctx.enter_context(tc.tile_pool(name="t2", bufs=4))
        op = ctx.enter_context(tc.tile_pool(name="o", bufs=3))

        wk_t = wkp.tile([P_DIM, Q], BF16)
        nc.sync.dma_start(out=wk_t[:], in_=Wk_d[:])

        PAIR = 2  # tiles fused per DVE op group (PSUM tile = 4 banks)
        for ch in range(N_CHUNKS):
            # chunk loads: CHUNK * 128 positions per DMA
            x_c = xp.tile([TILE_P, CHUNK, C_IN], BF16)
            nc.sync.dma_start(
                out=x_c[:],
                in_=X_d[bass.ts(ch, TILE_P * CHUNK), :].rearrange(
                    "(a p) i -> p a i", p=TILE_P
                ),
            )
            pT_c = pp.tile([P_DIM, CHUNK * TILE_P], BF16)
            nc.sync.dma_start(
                out=pT_c[:], in_=PT_d[:, bass.ts(ch, TILE_P * CHUNK)]
            )
            o_c = op.tile([TILE_P, CHUNK, C_OUT], BF16)

            for g in range(CHUNK // PAIR):
                a_t = apool.tile([TILE_P, PAIR, Q], F32)
                for j in range(PAIR):
                    lhsT = pT_c[:, bass.ts(g * PAIR + j, TILE_P)]
                    nc.tensor.matmul(
                        a_t[:, j, 0:512], lhsT=lhsT, rhs=wk_t[:, 0:512],
                        start=True, stop=True,
                    )
                    nc.tensor.matmul(
                        a_t[:, j, 512:1024], lhsT=lhsT, rhs=wk_t[:, 512:1024],
                        start=True, stop=True,
                    )

                # relu: PSUM -> SBUF, cast to bf16 (ACT engine), 2 tiles/op
                w_t = wp.tile([TILE_P, PAIR, Q], BF16)
                nc.scalar.activation(w_t[:], a_t[:], relu)

                # m[p, j, o, i] = w[p, j, o, i] * x[p, j, i]   (DVE, 2x bf16)
                m_t = mp.tile([TILE_P, PAIR, Q], BF16)
                w4 = w_t[:].rearrange("p j (o i) -> p j o i", o=C_OUT)
                m4 = m_t[:].rearrange("p j (o i) -> p j o i", o=C_OUT)
                x4 = x_c[:, bass.ts(g, PAIR), :].unsqueeze(2).broadcast_to(
                    [TILE_P, PAIR, C_OUT, C_IN]
                )
                nc.vector.tensor_tensor(out=m4, in0=w4, in1=x4, op=mult)

                # Reduce over i (innermost, 32 wide). TensorReduce has no
                # 2x DVE mode, so halve twice with 2x TENSOR_TENSOR adds,
                # then one short reduce.
                t1 = t1p.tile([TILE_P, PAIR, C_OUT, 16], BF16)
                nc.vector.tensor_tensor(
                    out=t1[:], in0=m4[:, :, :, 0:16], in1=m4[:, :, :, 16:32],
                    op=add,
                )
                t2 = t2p.tile([TILE_P, PAIR, C_OUT, 8], BF16)
                nc.vector.tensor_tensor(
                    out=t2[:], in0=t1[:, :, :, 0:8], in1=t1[:, :, :, 8:16],
                    op=add,
                )
                with nc.allow_low_precision("bf16 reduce, fp32 internal accum"):
                    nc.vector.tensor_reduce(
                        out=o_c[:, bass.ts(g, PAIR), :], in_=t2[:],
                        axis=mybir.AxisListType.X, op=add,
                    )

            nc.sync.dma_start(
                out=out_d[bass.ts(ch, TILE_P * CHUNK), :].rearrange(
                    "(a p) i -> p a i", p=TILE_P
                ),
                in_=o_c[:],
            )

    nc.finalize()
    return nc


def _get_nc():
    key = "v2"
    if key not in _BUILD_CACHE:
        _BUILD_CACHE[key] = _build_nc()
    return _BUILD_CACHE[key]


def kernel(X, P, Wk):
    global LAST_RESULTS
    X = np.asarray(X, dtype=np.float32)
    P = np.asarray(P, dtype=np.float32)
    Wk = np.asarray(Wk, dtype=np.float32)
    bf16 = ml_dtypes.bfloat16

    # Host-side prep (free): shard, transpose P, permute Wk columns so the
    # device-side layout is q = o*32 + i; cast matmul operands to bf16.
    WkP = np.ascontiguousarray(
        Wk.reshape(P_DIM, C_IN, C_OUT).transpose(0, 2, 1).reshape(P_DIM, Q)
    ).astype(bf16)
    in_maps = []
    for c in range(N_CORES):
        Xc = np.ascontiguousarray(
            X[c * B_SH:(c + 1) * B_SH].reshape(NPOS, C_IN)
        ).astype(bf16)
        PTc = np.ascontiguousarray(
            P[c * B_SH:(c + 1) * B_SH].reshape(NPOS, P_DIM).T
        ).astype(bf16)
        in_maps.append({"X": Xc, "PT": PTc, "Wk": WkP})

    nc = _get_nc()
    trace = os.environ.get("BASS_PROFILE", "0") == "1"
    kw = {}
    if os.environ.get("BASS_TMPDIR"):
        kw["tmpdir"] = os.environ["BASS_TMPDIR"]
    res = run_bass_kernel_spmd(
        nc, in_maps, list(range(N_CORES)), trace=trace, **kw
    )
    LAST_RESULTS = res

    out = np.empty((B, N, C_OUT), dtype=np.float32)
    for c in range(N_CORES):
        out[c * B_SH:(c + 1) * B_SH] = (
            np.asarray(res.results[c]["out"])
            .astype(np.float32)
            .reshape(B_SH, N, C_OUT)
        )
    return out


# revision 18
# speedup vs baseline: 1.5046x; 1.0136x over previous
"""Trainium2 Bass kernel for nn_ConditionedDense (hypernetwork-conditioned dense).

Reference computation:
    A = einsum('bnp,pq->bnq', P, Wk)         # hypernetwork: per-position weights
    W = relu(A).reshape(B, N, c_in, c_out)
    out = einsum('bni,bnio->bno', X, W)

Strategy: pure data parallel over 8 NeuronCores (shard batch dim). Per core
16384 positions, tiled 128 positions/tile, 4 tiles per DMA chunk:
  - PE matmul computes A-tile [128 pos, 1024] in PSUM (lhsT = P^T tile,
    rhs = Wk, both bf16; Wk host-permuted to q = o*32+i layout)
  - ACT applies relu (PSUM -> SBUF, bf16 out)
  - DVE (and GPSIMD for a fraction of tiles) multiplies by X broadcast
    over o; DVE grouped-reduces over i (innermost) and upcasts to fp32
Host side (free): P transposed per shard, Wk column-permuted, X/P/Wk cast
to bf16.
"""

import os
from contextlib import ExitStack

import numpy as np
import ml_dtypes

import concourse.bass as bass
import concourse.tile as tile
from concourse import bacc, mybir
from concourse.bass_utils import run_bass_kernel_spmd

C_IN = 32
C_OUT = 32
P_DIM = 64
Q = C_IN * C_OUT  # 1024
B, N = 32, 4096
N_CORES = 8
B_SH = B // N_CORES          # 4 batches per core
NPOS = B_SH * N              # 16384 positions per core
TILE_P = 128                 # positions per tile
N_TILES = NPOS // TILE_P     # 128
CHUNK = 8                    # tiles per DMA chunk
N_CHUNKS = N_TILES // CHUNK  # 16

F32 = mybir.dt.float32
BF16 = mybir.dt.bfloat16

_BUILD_CACHE = {}
LAST_RESULTS = None  # BassKernelResults of the most recent run (for profiling)


def _build_nc():
    nc = bacc.Bacc(
        "TRN2", target_bir_lowering=False, debug=False, num_devices=N_CORES
    )
    X_d = nc.declare_dram_parameter("X", [NPOS, C_IN], BF16, isOutput=False)
    PT_d = nc.declare_dram_parameter("PT", [P_DIM, NPOS], BF16, isOutput=False)
    Wk_d = nc.declare_dram_parameter("Wk", [P_DIM, Q], BF16, isOutput=False)
    out_d = nc.declare_dram_parameter("out", [NPOS, C_OUT], BF16, isOutput=True)

    relu = mybir.ActivationFunctionType.Relu
    mult = mybir.AluOpType.mult
    add = mybir.AluOpType.add

    with ExitStack() as ctx:
        tc = ctx.enter_context(tile.TileContext(nc))
        wkp = ctx.enter_context(tc.tile_pool(name="wk", bufs=1))
        xp = ctx.enter_context(tc.tile_pool(name="x", bufs=3))
        pp = ctx.enter_context(tc.tile_pool(name="pT", bufs=3))
        apool = ctx.enter_context(tc.tile_pool(name="apsum", bufs=2, space="PSUM"))
        wp = ctx.enter_context(tc.tile_pool(name="w", bufs=4))
        mp = ctx.enter_context(tc.tile_pool(name="m", bufs=4))
        t1p = ctx.enter_context(tc.tile_pool(name="t1", bufs=4))
        t2p = ctx.enter_context(tc.tile_pool(name="t2", bufs=4))
        op = ctx.enter_context(tc.tile_pool(name="o", bufs=3))

        wk_t = wkp.tile([P_DIM, Q], BF16)
        nc.sync.dma_start(out=wk_t[:], in_=Wk_d[:])

        PAIR = 2   # tiles per PSUM tile / ACT relu op (PSUM tile = 4 banks)
        GRP = 4    # tiles fused per DVE op group (w tile spans 2 relu outputs)
        for ch in range(N_CHUNKS):
            # chunk loads: CHUNK * 128 positions per DMA
            x_c = xp.tile([TILE_P, CHUNK, C_IN], BF16)
            nc.sync.dma_start(
                out=x_c[:],
                in_=X_d[bass.ts(ch, TILE_P * CHUNK), :].rearrange(
                    "(a p) i -> p a i", p=TILE_P
                ),
            )
            pT_c = pp.tile([P_DIM, CHUNK * TILE_P], BF16)
            nc.sync.dma_start(
                out=pT_c[:], in_=PT_d[:, bass.ts(ch, TILE_P * CHUNK)]
            )
            o_c = op.tile([TILE_P, CHUNK, C_OUT], BF16)

            for g in range(CHUNK // GRP):
                # w tile spans GRP tiles; filled by GRP//PAIR relu ops
                w_t = wp.tile([TILE_P, GRP, Q], BF16)
                for h in range(GRP // PAIR):
                    a_t = apool.tile([TILE_P, PAIR, Q], F32)
                    for j in range(PAIR):
                        lhsT = pT_c[
                            :, bass.ts(g * GRP + h * PAIR + j, TILE_P)
                        ]
                        nc.tensor.matmul(
                            a_t[:, j, 0:512], lhsT=lhsT, rhs=wk_t[:, 0:512],
                            start=True, stop=True,
                        )
                        nc.tensor.matmul(
                            a_t[:, j, 512:1024], lhsT=lhsT,
                            rhs=wk_t[:, 512:1024], start=True, stop=True,
                        )
                    # relu: PSUM -> SBUF, cast to bf16 (ACT), 2 tiles/op
                    nc.scalar.activation(
                        w_t[:, bass.ts(h, PAIR), :], a_t[:], relu
                    )

                # m[p, j, o, i] = w[p, j, o, i] * x[p, j, i]   (DVE, 2x bf16)
                m_t = mp.tile([TILE_P, GRP, Q], BF16)
                w4 = w_t[:].rearrange("p j (o i) -> p j o i", o=C_OUT)
                m4 = m_t[:].rearrange("p j (o i) -> p j o i", o=C_OUT)
                x4 = x_c[:, bass.ts(g, GRP), :].unsqueeze(2).broadcast_to(
                    [TILE_P, GRP, C_OUT, C_IN]
                )
                nc.vector.tensor_tensor(out=m4, in0=w4, in1=x4, op=mult)

                # Reduce over i (innermost, 32 wide). TensorReduce has no
                # 2x DVE mode, so halve twice with 2x TENSOR_TENSOR adds,
                # then one short reduce.
                t1 = t1p.tile([TILE_P, GRP, C_OUT, 16], BF16)
                nc.vector.tensor_tensor(
                    out=t1[:], in0=m4[:, :, :, 0:16], in1=m4[:, :, :, 16:32],
                    op=add,
                )
                t2 = t2p.tile([TILE_P, GRP, C_OUT, 8], BF16)
                nc.vector.tensor_tensor(
                    out=t2[:], in0=t1[:, :, :, 0:8], in1=t1[:, :, :, 8:16],
                    op=add,
                )
                with nc.allow_low_precision("bf16 reduce, fp32 internal accum"):
                    nc.vector.tensor_reduce(
                        out=o_c[:, bass.ts(g, GRP), :], in_=t2[:],
                        axis=mybir.AxisListType.X, op=add,
                    )

            nc.sync.dma_start(
                out=out_d[bass.ts(ch, TILE_P * CHUNK), :].rearrange(
                    "(a p) i -> p a i", p=TILE_P
                ),
                in_=o_c[:],
            )

    nc.finalize()
    return nc


def _get_nc():
    key = "v2"
    if key not in _BUILD_CACHE:
        _BUILD_CACHE[key] = _build_nc()
    return _BUILD_CACHE[key]


def kernel(X, P, Wk):
    global LAST_RESULTS
    X = np.asarray(X, dtype=np.float32)
    P = np.asarray(P, dtype=np.float32)
    Wk = np.asarray(Wk, dtype=np.float32)
    bf16 = ml_dtypes.bfloat16

    # Host-side prep (free): shard, transpose P, permute Wk columns so the
    # device-side layout is q = o*32 + i; cast matmul operands to bf16.
    WkP = np.ascontiguousarray(
        Wk.reshape(P_DIM, C_IN, C_OUT).transpose(0, 2, 1).reshape(P_DIM, Q)
    ).astype(bf16)
    in_maps = []
    for c in range(N_CORES):
        Xc = np.ascontiguousarray(
            X[c * B_SH:(c + 1) * B_SH].reshape(NPOS, C_IN)
        ).astype(bf16)
        PTc = np.ascontiguousarray(
            P[c * B_SH:(c + 1) * B_SH].reshape(NPOS, P_DIM).T
        ).astype(bf16)
        in_maps.append({"X": Xc, "PT": PTc, "Wk": WkP})

    nc = _get_nc()
    trace = os.environ.get("BASS_PROFILE", "0") == "1"
    kw = {}
    if os.environ.get("BASS_TMPDIR"):
        kw["tmpdir"] = os.environ["BASS_TMPDIR"]
    res = run_bass_kernel_spmd(
        nc, in_maps, list(range(N_CORES)), trace=trace, **kw
    )
    LAST_RESULTS = res

    out = np.empty((B, N, C_OUT), dtype=np.float32)
    for c in range(N_CORES):
        out[c * B_SH:(c + 1) * B_SH] = (
            np.asarray(res.results[c]["out"])
            .astype(np.float32)
            .reshape(B_SH, N, C_OUT)
        )
    return out


# revision 20
# speedup vs baseline: 1.5873x; 1.0550x over previous
"""Trainium2 Bass kernel for nn_ConditionedDense (hypernetwork-conditioned dense).

Reference computation:
    A = einsum('bnp,pq->bnq', P, Wk)         # hypernetwork: per-position weights
    W = relu(A).reshape(B, N, c_in, c_out)
    out = einsum('bni,bnio->bno', X, W)

Strategy: pure data parallel over 8 NeuronCores (shard batch dim). Per core
16384 positions, tiled 128 positions/tile, 4 tiles per DMA chunk:
  - PE matmul computes A-tile [128 pos, 1024] in PSUM (lhsT = P^T tile,
    rhs = Wk, both bf16; Wk host-permuted to q = o*32+i layout)
  - ACT applies relu (PSUM -> SBUF, bf16 out)
  - DVE (and GPSIMD for a fraction of tiles) multiplies by X broadcast
    over o; DVE grouped-reduces over i (innermost) and upcasts to fp32
Host side (free): P transposed per shard, Wk column-permuted, X/P/Wk cast
to bf16.
"""

import os
from contextlib import ExitStack

import numpy as np
import ml_dtypes

import concourse.bass as bass
import concourse.tile as tile
from concourse import bacc, mybir
from concourse.bass_utils import run_bass_kernel_spmd

C_IN = 32
C_OUT = 32
P_DIM = 64
Q = C_IN * C_OUT  # 1024
B, N = 32, 4096
N_CORES = 8
B_SH = B // N_CORES          # 4 batches per core
NPOS = B_SH * N              # 16384 positions per core
TILE_P = 128                 # positions per tile
N_TILES = NPOS // TILE_P     # 128
CHUNK = 8                    # tiles per DMA chunk
N_CHUNKS = N_TILES // CHUNK  # 16

F32 = mybir.dt.float32
BF16 = mybir.dt.bfloat16

_BUILD_CACHE = {}
LAST_RESULTS = None  # BassKernelResults of the most recent run (for profiling)


def _build_nc():
    nc = bacc.Bacc(
        "TRN2", target_bir_lowering=False, debug=False, num_devices=N_CORES
    )
    X_d = nc.declare_dram_parameter("X", [NPOS, C_IN], BF16, isOutput=False)
    PT_d = nc.declare_dram_parameter("PT", [P_DIM, NPOS], BF16, isOutput=False)
    Wk_d = nc.declare_dram_parameter("Wk", [P_DIM, Q], BF16, isOutput=False)
    out_d = nc.declare_dram_parameter("out", [NPOS, C_OUT], BF16, isOutput=True)

    relu = mybir.ActivationFunctionType.Relu
    mult = mybir.AluOpType.mult
    add = mybir.AluOpType.add

    with ExitStack() as ctx:
        tc = ctx.enter_context(tile.TileContext(nc))
        wkp = ctx.enter_context(tc.tile_pool(name="wk", bufs=1))
        xp = ctx.enter_context(tc.tile_pool(name="x", bufs=3))
        pp = ctx.enter_context(tc.tile_pool(name="pT", bufs=3))
        apool = ctx.enter_context(tc.tile_pool(name="apsum", bufs=2, space="PSUM"))
        wp = ctx.enter_context(tc.tile_pool(name="w", bufs=4))
        mp = ctx.enter_context(tc.tile_pool(name="m", bufs=4))
        t1p = ctx.enter_context(tc.tile_pool(name="t1", bufs=4))
        t2p = ctx.enter_context(tc.tile_pool(name="t2", bufs=4))
        t3p = ctx.enter_context(tc.tile_pool(name="t3", bufs=4))
        op = ctx.enter_context(tc.tile_pool(name="o", bufs=3))

        wk_t = wkp.tile([P_DIM, Q], BF16)
        nc.sync.dma_start(out=wk_t[:], in_=Wk_d[:])

        PAIR = 2   # tiles per PSUM tile / ACT relu op (PSUM tile = 4 banks)
        GRP = 4    # tiles fused per DVE op group (w tile spans 2 relu outputs)
        for ch in range(N_CHUNKS):
            # chunk loads: CHUNK * 128 positions per DMA
            x_c = xp.tile([TILE_P, CHUNK, C_IN], BF16)
            nc.sync.dma_start(
                out=x_c[:],
                in_=X_d[bass.ts(ch, TILE_P * CHUNK), :].rearrange(
                    "(a p) i -> p a i", p=TILE_P
                ),
            )
            pT_c = pp.tile([P_DIM, CHUNK * TILE_P], BF16)
            nc.sync.dma_start(
                out=pT_c[:], in_=PT_d[:, bass.ts(ch, TILE_P * CHUNK)]
            )
            o_c = op.tile([TILE_P, CHUNK, C_OUT], BF16)

            for g in range(CHUNK // GRP):
                # w tile spans GRP tiles; filled by GRP//PAIR relu ops
                w_t = wp.tile([TILE_P, GRP, Q], BF16)
                for h in range(GRP // PAIR):
                    a_t = apool.tile([TILE_P, PAIR, Q], F32)
                    for j in range(PAIR):
                        lhsT = pT_c[
                            :, bass.ts(g * GRP + h * PAIR + j, TILE_P)
                        ]
                        nc.tensor.matmul(
                            a_t[:, j, 0:512], lhsT=lhsT, rhs=wk_t[:, 0:512],
                            start=True, stop=True,
                        )
                        nc.tensor.matmul(
                            a_t[:, j, 512:1024], lhsT=lhsT,
                            rhs=wk_t[:, 512:1024], start=True, stop=True,
                        )
                    # relu: PSUM -> SBUF, cast to bf16 (ACT), 2 tiles/op
                    nc.scalar.activation(
                        w_t[:, bass.ts(h, PAIR), :], a_t[:], relu
                    )

                # m[p, j, o, i] = w[p, j, o, i] * x[p, j, i]   (DVE, 2x bf16)
                m_t = mp.tile([TILE_P, GRP, Q], BF16)
                w4 = w_t[:].rearrange("p j (o i) -> p j o i", o=C_OUT)
                m4 = m_t[:].rearrange("p j (o i) -> p j o i", o=C_OUT)
                x4 = x_c[:, bass.ts(g, GRP), :].unsqueeze(2).broadcast_to(
                    [TILE_P, GRP, C_OUT, C_IN]
                )
                nc.vector.tensor_tensor(out=m4, in0=w4, in1=x4, op=mult)

                # Reduce over i (innermost, 32 wide). TensorReduce has no
                # 2x DVE mode, so halve twice with 2x TENSOR_TENSOR adds,
                # then one short reduce.
                t1 = t1p.tile([TILE_P, GRP, C_OUT, 16], BF16)
                nc.vector.tensor_tensor(
                    out=t1[:], in0=m4[:, :, :, 0:16], in1=m4[:, :, :, 16:32],
                    op=add,
                )
                t2 = t2p.tile([TILE_P, GRP, C_OUT, 8], BF16)
                nc.vector.tensor_tensor(
                    out=t2[:], in0=t1[:, :, :, 0:8], in1=t1[:, :, :, 8:16],
                    op=add,
                )
                t3 = t3p.tile([TILE_P, GRP, C_OUT, 4], BF16)
                nc.vector.tensor_tensor(
                    out=t3[:], in0=t2[:, :, :, 0:4], in1=t2[:, :, :, 4:8],
                    op=add,
                )
                with nc.allow_low_precision("bf16 reduce, fp32 internal accum"):
                    nc.vector.tensor_reduce(
                        out=o_c[:, bass.ts(g, GRP), :], in_=t3[:],
                        axis=mybir.AxisListType.X, op=add,
                    )

            nc.sync.dma_start(
                out=out_d[bass.ts(ch, TILE_P * CHUNK), :].rearrange(
                    "(a p) i -> p a i", p=TILE_P
                ),
                in_=o_c[:],
            )

    nc.finalize()
    return nc


def _get_nc():
    key = "v2"
    if key not in _BUILD_CACHE:
        _BUILD_CACHE[key] = _build_nc()
    return _BUILD_CACHE[key]


def kernel(X, P, Wk):
    global LAST_RESULTS
    X = np.asarray(X, dtype=np.float32)
    P = np.asarray(P, dtype=np.float32)
    Wk = np.asarray(Wk, dtype=np.float32)
    bf16 = ml_dtypes.bfloat16

    # Host-side prep (free): shard, transpose P, permute Wk columns so the
    # device-side layout is q = o*32 + i; cast matmul operands to bf16.
    WkP = np.ascontiguousarray(
        Wk.reshape(P_DIM, C_IN, C_OUT).transpose(0, 2, 1).reshape(P_DIM, Q)
    ).astype(bf16)
    in_maps = []
    for c in range(N_CORES):
        Xc = np.ascontiguousarray(
            X[c * B_SH:(c + 1) * B_SH].reshape(NPOS, C_IN)
        ).astype(bf16)
        PTc = np.ascontiguousarray(
            P[c * B_SH:(c + 1) * B_SH].reshape(NPOS, P_DIM).T
        ).astype(bf16)
        in_maps.append({"X": Xc, "PT": PTc, "Wk": WkP})

    nc = _get_nc()
    trace = os.environ.get("BASS_PROFILE", "0") == "1"
    kw = {}
    if os.environ.get("BASS_TMPDIR"):
        kw["tmpdir"] = os.environ["BASS_TMPDIR"]
    res = run_bass_kernel_spmd(
        nc, in_maps, list(range(N_CORES)), trace=trace, **kw
    )
    LAST_RESULTS = res

    out = np.empty((B, N, C_OUT), dtype=np.float32)
    for c in range(N_CORES):
        out[c * B_SH:(c + 1) * B_SH] = (
            np.asarray(res.results[c]["out"])
            .astype(np.float32)
            .reshape(B_SH, N, C_OUT)
        )
    return out


# revision 21
# speedup vs baseline: 1.5959x; 1.0054x over previous
"""Trainium2 Bass kernel for nn_ConditionedDense (hypernetwork-conditioned dense).

Reference computation:
    A = einsum('bnp,pq->bnq', P, Wk)         # hypernetwork: per-position weights
    W = relu(A).reshape(B, N, c_in, c_out)
    out = einsum('bni,bnio->bno', X, W)

Strategy: pure data parallel over 8 NeuronCores (shard batch dim). Per core
16384 positions, tiled 128 positions/tile, 4 tiles per DMA chunk:
  - PE matmul computes A-tile [128 pos, 1024] in PSUM (lhsT = P^T tile,
    rhs = Wk, both bf16; Wk host-permuted to q = o*32+i layout)
  - ACT applies relu (PSUM -> SBUF, bf16 out)
  - DVE (and GPSIMD for a fraction of tiles) multiplies by X broadcast
    over o; DVE grouped-reduces over i (innermost) and upcasts to fp32
Host side (free): P transposed per shard, Wk column-permuted, X/P/Wk cast
to bf16.
"""

import os
from contextlib import ExitStack

import numpy as np
import ml_dtypes

import concourse.bass as bass
import concourse.tile as tile
from concourse import bacc, mybir
from concourse.bass_utils import run_bass_kernel_spmd

C_IN = 32
C_OUT = 32
P_DIM = 64
Q = C_IN * C_OUT  # 1024
B, N = 32, 4096
N_CORES = 8
B_SH = B // N_CORES          # 4 batches per core
NPOS = B_SH * N              # 16384 positions per core
TILE_P = 128                 # positions per tile
N_TILES = NPOS // TILE_P     # 128
CHUNK = 8                    # tiles per DMA chunk
N_CHUNKS = N_TILES // CHUNK  # 16

F32 = mybir.dt.float32
BF16 = mybir.dt.bfloat16

_BUILD_CACHE = {}
LAST_RESULTS = None  # BassKernelResults of the most recent run (for profiling)


def _build_nc():
    nc = bacc.Bacc(
        "TRN2", target_bir_lowering=False, debug=False, num_devices=N_CORES
    )
    X_d = nc.declare_dram_parameter("X", [NPOS, C_IN], BF16, isOutput=False)
    PT_d = nc.declare_dram_parameter("PT", [P_DIM, NPOS], BF16, isOutput=False)
    Wk_d = nc.declare_dram_parameter("Wk", [P_DIM, Q], BF16, isOutput=False)
    out_d = nc.declare_dram_parameter("out", [NPOS, C_OUT], BF16, isOutput=True)

    relu = mybir.ActivationFunctionType.Relu
    mult = mybir.AluOpType.mult
    add = mybir.AluOpType.add

    with ExitStack() as ctx:
        tc = ctx.enter_context(tile.TileContext(nc))
        wkp = ctx.enter_context(tc.tile_pool(name="wk", bufs=1))
        xp = ctx.enter_context(tc.tile_pool(name="x", bufs=3))
        pp = ctx.enter_context(tc.tile_pool(name="pT", bufs=3))
        apool = ctx.enter_context(tc.tile_pool(name="apsum", bufs=2, space="PSUM"))
        wp = ctx.enter_context(tc.tile_pool(name="w", bufs=2))
        mp = ctx.enter_context(tc.tile_pool(name="m", bufs=2))
        t1p = ctx.enter_context(tc.tile_pool(name="t1", bufs=2))
        t2p = ctx.enter_context(tc.tile_pool(name="t2", bufs=2))
        t3p = ctx.enter_context(tc.tile_pool(name="t3", bufs=2))
        op = ctx.enter_context(tc.tile_pool(name="o", bufs=3))

        wk_t = wkp.tile([P_DIM, Q], BF16)
        nc.sync.dma_start(out=wk_t[:], in_=Wk_d[:])

        PAIR = 2   # tiles per PSUM tile / ACT relu op (PSUM tile = 4 banks)
        GRP = 8    # tiles fused per DVE op group (w tile spans 4 relu outputs)
        for ch in range(N_CHUNKS):
            # chunk loads: CHUNK * 128 positions per DMA
            x_c = xp.tile([TILE_P, CHUNK, C_IN], BF16)
            nc.sync.dma_start(
                out=x_c[:],
                in_=X_d[bass.ts(ch, TILE_P * CHUNK), :].rearrange(
                    "(a p) i -> p a i", p=TILE_P
                ),
            )
            pT_c = pp.tile([P_DIM, CHUNK * TILE_P], BF16)
            nc.sync.dma_start(
                out=pT_c[:], in_=PT_d[:, bass.ts(ch, TILE_P * CHUNK)]
            )
            o_c = op.tile([TILE_P, CHUNK, C_OUT], BF16)

            for g in range(CHUNK // GRP):
                # w tile spans GRP tiles; filled by GRP//PAIR relu ops
                w_t = wp.tile([TILE_P, GRP, Q], BF16)
                for h in range(GRP // PAIR):
                    a_t = apool.tile([TILE_P, PAIR, Q], F32)
                    for j in range(PAIR):
                        lhsT = pT_c[
                            :, bass.ts(g * GRP + h * PAIR + j, TILE_P)
                        ]
                        nc.tensor.matmul(
                            a_t[:, j, 0:512], lhsT=lhsT, rhs=wk_t[:, 0:512],
                            start=True, stop=True,
                        )
                        nc.tensor.matmul(
                            a_t[:, j, 512:1024], lhsT=lhsT,
                            rhs=wk_t[:, 512:1024], start=True, stop=True,
                        )
                    # relu: PSUM -> SBUF, cast to bf16 (ACT), 2 tiles/op
                    nc.scalar.activation(
                        w_t[:, bass.ts(h, PAIR), :], a_t[:], relu
                    )

                # m[p, j, o, i] = w[p, j, o, i] * x[p, j, i]   (DVE, 2x bf16)
                m_t = mp.tile([TILE_P, GRP, Q], BF16)
                w4 = w_t[:].rearrange("p j (o i) -> p j o i", o=C_OUT)
                m4 = m_t[:].rearrange("p j (o i) -> p j o i", o=C_OUT)
                x4 = x_c[:, bass.ts(g, GRP), :].unsqueeze(2).broadcast_to(
                    [TILE_P, GRP, C_OUT, C_IN]
                )
                nc.vector.tensor_tensor(out=m4, in0=w4, in1=x4, op=mult)

                # Reduce over i (innermost, 32 wide). TensorReduce has no
                # 2x DVE mode, so halve twice with 2x TENSOR_TENSOR adds,
                # then one short reduce.
                t1 = t1p.tile([TILE_P, GRP, C_OUT, 16], BF16)
                nc.vector.tensor_tensor(
                    out=t1[:], in0=m4[:, :, :, 0:16], in1=m4[:, :, :, 16:32],
                    op=add,
                )
                t2 = t2p.tile([TILE_P, GRP, C_OUT, 8], BF16)
                nc.vector.tensor_tensor(
                    out=t2[:], in0=t1[:, :, :, 0:8], in1=t1[:, :, :, 8:16],
                    op=add,
                )
                t3 = t3p.tile([TILE_P, GRP, C_OUT, 4], BF16)
                nc.vector.tensor_tensor(
                    out=t3[:], in0=t2[:, :, :, 0:4], in1=t2[:, :, :, 4:8],
                    op=add,
                )
                with nc.allow_low_precision("bf16 reduce, fp32 internal accum"):
                    nc.vector.tensor_reduce(
                        out=o_c[:, bass.ts(g, GRP), :], in_=t3[:],
                        axis=mybir.AxisListType.X, op=add,
                    )

            nc.sync.dma_start(
                out=out_d[bass.ts(ch, TILE_P * CHUNK), :].rearrange(
                    "(a p) i -> p a i", p=TILE_P
                ),
                in_=o_c[:],
            )

    nc.finalize()
    return nc


def _get_nc():
    key = "v2"
    if key not in _BUILD_CACHE:
        _BUILD_CACHE[key] = _build_nc()
    return _BUILD_CACHE[key]


def kernel(X, P, Wk):
    global LAST_RESULTS
    X = np.asarray(X, dtype=np.float32)
    P = np.asarray(P, dtype=np.float32)
    Wk = np.asarray(Wk, dtype=np.float32)
    bf16 = ml_dtypes.bfloat16

    # Host-side prep (free): shard, transpose P, permute Wk columns so the
    # device-side layout is q = o*32 + i; cast matmul operands to bf16.
    WkP = np.ascontiguousarray(
        Wk.reshape(P_DIM, C_IN, C_OUT).transpose(0, 2, 1).reshape(P_DIM, Q)
    ).astype(bf16)
    in_maps = []
    for c in range(N_CORES):
        Xc = np.ascontiguousarray(
            X[c * B_SH:(c + 1) * B_SH].reshape(NPOS, C_IN)
        ).astype(bf16)
        PTc = np.ascontiguousarray(
            P[c * B_SH:(c + 1) * B_SH].reshape(NPOS, P_DIM).T
        ).astype(bf16)
        in_maps.append({"X": Xc, "PT": PTc, "Wk": WkP})

    nc = _get_nc()
    trace = os.environ.get("BASS_PROFILE", "0") == "1"
    kw = {}
    if os.environ.get("BASS_TMPDIR"):
        kw["tmpdir"] = os.environ["BASS_TMPDIR"]
    res = run_bass_kernel_spmd(
        nc, in_maps, list(range(N_CORES)), trace=trace, **kw
    )
    LAST_RESULTS = res

    out = np.empty((B, N, C_OUT), dtype=np.float32)
    for c in range(N_CORES):
        out[c * B_SH:(c + 1) * B_SH] = (
            np.asarray(res.results[c]["out"])
            .astype(np.float32)
            .reshape(B_SH, N, C_OUT)
        )
    return out
